# revision 65
# baseline (speedup 1.0000x reference)
"""Trainium2 Bass kernel for nn_Attention_spd (dense transformer attention
with spd-modulated bias), sharded batch-parallel across 8 NeuronCores.

Per batch b (one core each):
    qkv = x @ W_qkv ; q,k,v heads of 64
    dots = q @ k^T * DH**-0.5                       [h, n, m]
    bias = normalize(dots*spd, axis=-1) * ||dots||  (row-wise L2)
    attn = softmax(dots + bias) * head_mask_scale
    out  = (attn @ v) merged @ W_out + b_out

Identities used on device:
    z    = dots * (1 + spd * rho[n]),  rho = ||dots|| / max(||dots*spd||, eps)
    u[n] = sum_m dots^2   via ACT Square accum_out (rides the d2 pass)
    ssq  = sum_m d2 * spd^2  via DVE STT accum_out (spd^2 from host)
    rho  = exp(0.5*(ln(u+eps) - ln(ssq+eps)))  (single ACT table set)
    softmax denominator via ACT-exp accumulate; 1/den applied to E.

Structure: dots for each (head, n-tile) is computed ONCE into PSUM and
immediately copied to fp16 SBUF (copy duty alternates DVE/ACT to balance
load), freeing the PSUM bank so the PE never stalls. A software pipeline
emits stage A (dots/copy/d2/ssq) 4 tiles ahead of stage B1 (rho/w/z/exp)
and 6 ahead of B2 (1/den scale + transpose + EV), so each in-order engine
queue always holds independent big ops ahead of the cross-engine small-op
chains. Steady state runs the Vector engine at ~100% occupancy.

Precision plan: fp16 x/Wqk/spd and fp16 matmuls everywhere (1 cyc/col on
the PE); dots accumulated in fp32 PSUM; E in bf16 (range safety); E/v
matmul bf16. Host folds DH**-0.5 into the k columns of W_qkv and
head_mask * H/sum(mask) into the rows of W_out; x passed pre-transposed,
spd passed with its elementwise square.
"""

import numpy as np
from contextlib import ExitStack

import concourse.bass as bass
import concourse.tile as tile
import concourse.mybir as mybir
from concourse.bass_utils import run_bass_kernel_spmd
from concourse.vector_clock import ScopedClock

# ---------------- problem constants (hardcoded) ----------------
B, N, DIM, H, DH = 8, 1024, 512, 8, 64
INNER = H * DH            # 512
SCALE = DH ** -0.5
P = 128                   # SBUF partitions
NT = N // P               # 8 n-tiles (and m-tiles)
KD = DIM // P             # 4 contraction subtiles over DIM
AF = mybir.ActivationFunctionType
ALU = mybir.AluOpType
F32 = mybir.dt.float32
F16 = mybir.dt.float16
BF16 = mybir.dt.bfloat16

# ---------------- walrus workaround patches ----------------
# The walrus build in this container rejects instructions with more than one
# sync-wait command. Split excess waits onto same-engine NoOps, and spread
# the kernel-tail Drain's waits over extra Drains.
_MAX_WAITS = 1
_SKIP_TYPES = (
    "TileBranchInst",
    "BassTileLoopBlock",
    "BassTileConditionalBlock",
    "BassTileCriticalSection",
)


def _split_waits(nc, ordered):
    for _bb, insts in ordered.items():
        new_list = []
        for inst in insts:
            si = getattr(inst, "sync_info", None)
            if (
                si is not None
                and si.on_wait
                and len(si.on_wait) > _MAX_WAITS
                and type(inst).__name__ not in _SKIP_TYPES
            ):
                waits = list(si.on_wait)
                extra, keep = waits[:-_MAX_WAITS], waits[-_MAX_WAITS:]
                for j in range(0, len(extra), _MAX_WAITS):
                    nop = mybir.InstNoOp(
                        name=nc.get_next_instruction_name(),
                        sync_info=mybir.SyncInfo(
                            on_wait=extra[j : j + _MAX_WAITS], on_update=[]
                        ),
                        bass_nofuse=True,
                        engine=inst.engine,
                    )
                    new_list.append(nop)
                inst.sync_info = mybir.SyncInfo(on_wait=keep, on_update=si.on_update)
            new_list.append(inst)
        insts[:] = new_list


_orig_lower = tile.TileContext._lower_ordered_insts


def _patched_lower(self, ordered):
    _split_waits(self.nc, ordered)
    return _orig_lower(self, ordered)


def _patched_drain_and_barrier(self, tick_clock, wait_clock):
    nc = self.nc
    drain_inst = nc.sync.drain()
    wait_clock.add_sem_waits(
        drain_inst.ins, ScopedClock({None: tick_clock.global_clock})
    )
    waits = list(drain_inst.ins.sync_info.on_wait)
    if len(waits) > 1:
        drain_inst.ins.sync_info = mybir.SyncInfo(on_wait=waits[:1], on_update=[])
        for w in waits[1:]:
            extra = nc.sync.drain()
            extra.ins.sync_info = mybir.SyncInfo(on_wait=[w], on_update=[])
    nc.all_engine_barrier()
    assert self.sems is not None
    popped = nc._tile_sem_poison_stack.pop()
    assert popped is self._sem_poison
    nc.clear_and_free_semaphores(list(self.sems.allocated().values()))
    nc.all_engine_barrier()


def _apply_patches():
    if tile.TileContext._lower_ordered_insts is not _patched_lower:
        tile.TileContext._lower_ordered_insts = _patched_lower
        tile.TileContext._drain_and_barrier = _patched_drain_and_barrier


# ---------------- device kernel ----------------

def _build_bass():
    _apply_patches()
    nc = bass.Bass(
        "TRN2", target_bir_lowering=False, debug=False, enable_asserts=False
    )
    xt = nc.dram_tensor("xt", (DIM, N), F16, kind="ExternalInput").ap()
    spd = nc.dram_tensor("spd", (N, N), F16, kind="ExternalInput").ap()
    spd2 = nc.dram_tensor("spd2", (N, N), F16, kind="ExternalInput").ap()
    wqk = nc.dram_tensor("wqk", (DIM, 2 * INNER), F16, kind="ExternalInput").ap()
    wv = nc.dram_tensor("wv", (DIM, INNER), F16, kind="ExternalInput").ap()
    wout = nc.dram_tensor("wout", (INNER, DIM), F16, kind="ExternalInput").ap()
    bout = nc.dram_tensor("bout", (1, DIM), F16, kind="ExternalInput").ap()
    ones = nc.dram_tensor("ones", (1, P), F16, kind="ExternalInput").ap()
    y = nc.dram_tensor("y", (N, DIM), F32, kind="ExternalOutput").ap()

    with tile.TileContext(nc) as tc, ExitStack() as ctx:
        _emit(nc, tc, ctx, xt, spd, spd2, wqk, wv, wout, bout, ones, y)
    return nc


def _emit(nc, tc, ctx, xt, spd, spd2, wqk, wv, wout, bout, ones, y):
    # ------- persistent pools -------
    const_p = ctx.enter_context(tc.tile_pool(name="const", bufs=1))
    spd_p = ctx.enter_context(tc.tile_pool(name="spd", bufs=1))
    spd2_p = ctx.enter_context(tc.tile_pool(name="spd2", bufs=1))
    qkT_p = ctx.enter_context(tc.tile_pool(name="qkT", bufs=1))
    v_p = ctx.enter_context(tc.tile_pool(name="v", bufs=1))
    wout_p = ctx.enter_context(tc.tile_pool(name="wout", bufs=1))
    merged_p = ctx.enter_context(tc.tile_pool(name="merged", bufs=1))
    st_p = ctx.enter_context(tc.tile_pool(name="stats", bufs=1))

    ones1 = const_p.tile([1, P], F16)
    nc.sync.dma_start(ones1[:], ones[:])
    bout_sb = const_p.tile([1, DIM], F16)
    nc.sync.dma_start(bout_sb[:], bout[:])
    eps_t = const_p.tile([P, 1], F32, tag="eps")
    nc.vector.memset(eps_t[:], 1e-24)

    spd_sb = spd_p.tile([P, NT, N], F16)
    spd2_sb = spd2_p.tile([P, NT, N], F16)

    # qkT_sb[p, ft, n] = (x @ Wqk)^T : f = ft*128+p ; q is ft 0..3, k is ft 4..7
    qkT_sb = qkT_p.tile([P, 2 * INNER // P, N], F16)
    # v_sb[p, mt, :] = v[m, :] with m = mt*128+p (bf16 for the EV matmul)
    v_sb = v_p.tile([P, NT, INNER], BF16)
    wout_sb = wout_p.tile([P, KD, DIM], F16)
    nc.sync.dma_start(wout_sb[:], wout.rearrange("(kt p) d -> p kt d", p=P))
    # mergedT[p, s, n]: inner = s*128 + p  (head pair s = (2s, 2s+1))
    mergedT = merged_p.tile([P, KD, N], F16)

    # per-(head, n-tile) row stats; index hi = h*NT + i.
    # ssq (slot 0) and u (slot 1) interleaved so one Ln covers a pair.
    su_all = st_p.tile([P, H * NT, 2], F32, tag="su")
    lsu_all = st_p.tile([P, H * NT, 2], F32, tag="lsu")
    rho_all = st_p.tile([P, H * NT], F32, tag="rho")
    den_all = st_p.tile([P, H * NT], F32, tag="den")
    rden_all = st_p.tile([P, H * NT], F32, tag="rden")

    # ------- stage A: projections (all fp16 matmuls) -------
    xt_p = ctx.enter_context(tc.tile_pool(name="xt", bufs=1))
    wqk_p = ctx.enter_context(tc.tile_pool(name="wqk", bufs=1))
    wv_p = ctx.enter_context(tc.tile_pool(name="wv", bufs=1))

    xt_sb = xt_p.tile([P, KD, N], F16)
    nc.sync.dma_start(xt_sb[:], xt.rearrange("(kt p) n -> p kt n", p=P))
    wqk_sb = wqk_p.tile([P, KD, 2 * INNER], F16)
    nc.sync.dma_start(wqk_sb[:], wqk.rearrange("(kt p) f -> p kt f", p=P))
    wv_sb = wv_p.tile([P, KD, INNER], F16)
    # spd2/spd after the projection inputs: first needed at ssq/w of tile 0;
    # wv only at the first EV (~16 tiles in)
    nc.sync.dma_start(spd2_sb[:], spd2.rearrange("(t p) m -> p t m", p=P))
    nc.sync.dma_start(spd_sb[:], spd.rearrange("(t p) m -> p t m", p=P))
    nc.sync.dma_start(wv_sb[:], wv.rearrange("(kt p) f -> p kt f", p=P))

    _cpy = [0]

    def emit_qkT(ft_list, pool, width):
        for ft in ft_list:
            for ch in range(2):
                ps = pool.tile([P, width], F32, tag=None if width == 512 else "psd")
                for kt in range(KD):
                    nc.tensor.matmul(
                        ps[:, 0:512],
                        wqk_sb[:, kt, ft * P : (ft + 1) * P],
                        xt_sb[:, kt, ch * 512 : (ch + 1) * 512],
                        start=(kt == 0),
                        stop=(kt == KD - 1),
                    )
                dst = qkT_sb[:, ft, ch * 512 : (ch + 1) * 512]
                if _cpy[0] % 2 == 0:
                    nc.vector.tensor_copy(dst, ps[:, 0:512])
                else:
                    nc.scalar.copy(dst, ps[:, 0:512])
                _cpy[0] += 1

    def emit_vproj(pool, width):
        for mt in range(NT):
            ps = pool.tile([P, width], F32, tag=None if width == 512 else "psd")
            for kt in range(KD):
                nc.tensor.matmul(
                    ps[:, 0:512],
                    xt_sb[:, kt, mt * P : (mt + 1) * P],
                    wv_sb[:, kt, :],
                    start=(kt == 0),
                    stop=(kt == KD - 1),
                )
            if mt % 2 == 0:
                nc.vector.tensor_copy(v_sb[:, mt, :], ps[:, 0:512])
            else:
                nc.scalar.copy(v_sb[:, mt, :], ps[:, 0:512])

    # heads 0-1 (score tiles 0-15) projections up front; the rest interleave
    # into the score pipeline so the first dots aren't queued behind them
    with tc.tile_pool(name="ps_proj", bufs=4, space="PSUM") as ps_proj0:
        emit_qkT((0, 4, 1, 5), ps_proj0, 512)

    # ------- stage B: scores per head, single dots pass -------
    score_ctx = ExitStack()
    ctx.enter_context(score_ctx)
    et_p = score_ctx.enter_context(tc.tile_pool(name="ET", bufs=2))
    dc_p = score_ctx.enter_context(tc.tile_pool(name="dc", bufs=10))
    d2_p = score_ctx.enter_context(tc.tile_pool(name="d2", bufs=5))
    junk_p = score_ctx.enter_context(tc.tile_pool(name="sjunk", bufs=2))
    w_p = score_ctx.enter_context(tc.tile_pool(name="wbuf", bufs=5))
    z_p = score_ctx.enter_context(tc.tile_pool(name="zbuf", bufs=5))
    e_p = score_ctx.enter_context(tc.tile_pool(name="E", bufs=8))
    ln_p = score_ctx.enter_context(tc.tile_pool(name="lnjunk", bufs=4))
    rd_p = score_ctx.enter_context(tc.tile_pool(name="rden", bufs=2))
    ps_dots = score_ctx.enter_context(
        tc.tile_pool(name="ps_dots", bufs=3, space="PSUM")
    )
    ps_ev = score_ctx.enter_context(tc.tile_pool(name="ps_ev", bufs=2, space="PSUM"))

    et_tiles = {}
    dcs = {}
    zbufs = {}
    ebufs = {}

    def qk_mm(psd, h, i):
        base = (h % 2) * DH
        for ch in range(2):
            nc.tensor.matmul(
                psd[:, ch * 512 : (ch + 1) * 512],
                qkT_sb[base : base + DH, h // 2, i * P : (i + 1) * P],
                qkT_sb[base : base + DH, 4 + h // 2, ch * 512 : (ch + 1) * 512],
                start=True,
                stop=True,
            )

    TILES = [(h, i) for h in range(H) for i in range(NT)]

    def emit_A(t):
        h, i = TILES[t]
        hi = h * NT + i
        psd = ps_dots.tile([P, N], F32, tag="psd")
        qk_mm(psd, h, i)
        # copy dots out of PSUM right away (frees the bank for the PE);
        # alternate the PSUM-read duty between DVE and ACT to balance load
        dc = dc_p.tile([P, N], F16)
        dcs[t] = dc
        d2 = d2_p.tile([P, N], F16)
        if t % 2 == 0:
            nc.scalar.copy(dc[:], psd[:])
            nc.scalar.activation(
                d2[:], psd[:], AF.Square, accum_out=su_all[:, hi, 1:2]
            )
        else:
            nc.vector.tensor_copy(dc[:], psd[:])
            nc.scalar.activation(
                d2[:], dc[:], AF.Square, accum_out=su_all[:, hi, 1:2]
            )
        junk = junk_p.tile([P, N], F16)
        nc.vector.scalar_tensor_tensor(
            junk[:],
            d2[:],
            1.0,
            spd2_sb[:, i, :],
            ALU.mult,
            ALU.mult,
            accum_out=su_all[:, hi, 0:1],
        )

    GRP = 2

    def emit_B1(t0):
        # rho for the pair (t0, t0+1), then w, z, exp+den
        h, i0 = TILES[t0]
        lo = h * NT + i0
        sl = slice(lo, lo + GRP)
        nc.scalar.activation(
            lsu_all[:, sl, :], su_all[:, sl, :], AF.Ln, bias=eps_t[:, 0:1]
        )
        s4 = ln_p.tile([P, GRP], F32, tag="s4")
        nc.vector.tensor_tensor(
            s4[:], lsu_all[:, sl, 1], lsu_all[:, sl, 0], ALU.subtract
        )
        nc.scalar.activation(rho_all[:, sl], s4[:], AF.Exp, scale=0.5)
        for t in range(t0, t0 + GRP):
            h, i = TILES[t]
            hi = h * NT + i
            w16 = w_p.tile([P, N], F16)
            nc.vector.tensor_scalar(
                w16[:],
                spd_sb[:, i, :],
                rho_all[:, hi : hi + 1],
                1.0,
                ALU.mult,
                ALU.add,
            )
            z16 = z_p.tile([P, N], F16)
            nc.vector.tensor_tensor(z16[:], dcs.pop(t)[:], w16[:], ALU.mult)
            zbufs[t] = z16
        for t in range(t0, t0 + GRP):
            h, i = TILES[t]
            hi = h * NT + i
            eb = e_p.tile([P, N], BF16)
            ebufs[t] = eb
            nc.scalar.activation(
                eb[:], zbufs.pop(t)[:], AF.Exp, accum_out=den_all[:, hi : hi + 1]
            )

    def emit_B2(t0):
        # transpose raw (unnormalized) E out; EV when a head pair closes.
        # 1/den is folded into the EV psum->merged multiply via R.
        h, i0 = TILES[t0]
        et = et_tiles.get(h)
        if et is None:
            et = et_p.tile([P, NT, N], BF16)
            et_tiles[h] = et
        lo = h * NT + i0
        sl = slice(lo, lo + GRP)
        nc.vector.reciprocal(rden_all[:, sl], den_all[:, sl])
        for t in range(t0, t0 + GRP):
            _, i = TILES[t]
            hi = h * NT + i
            eb = ebufs.pop(t)
            nc.vector.tensor_scalar_mul(eb[:], eb[:], rden_all[:, hi : hi + 1])
            nc.sync.dma_start_transpose(et[:, :, i * P : (i + 1) * P], eb[:])
        if i0 + GRP == NT and h % 2 == 1:
            for ch in range(2):
                ps = ps_ev.tile([P, 512], F32)
                for hh in (h - 1, h):
                    bb = (hh % 2) * DH
                    for mt in range(NT):
                        nc.tensor.matmul(
                            ps[bb : bb + DH, :],
                            v_sb[:, mt, hh * DH : (hh + 1) * DH],
                            et_tiles[hh][:, mt, ch * 512 : (ch + 1) * 512],
                            start=(mt == 0),
                            stop=(mt == NT - 1),
                            tile_position=(0, bb),
                        )
                nc.scalar.copy(
                    mergedT[:, h // 2, ch * 512 : (ch + 1) * 512], ps[:]
                )
            et_tiles.clear()

    # software pipeline: A runs 2 tiles ahead of B1, 4 ahead of B2, so each
    # engine's in-order queue always holds independent big ops ahead of the
    # cross-engine rho/den small-op chains.
    NTILES = len(TILES)
    for idx in range(NTILES + 7):
        if idx < NTILES:
            emit_A(idx)
        if idx == 2:
            emit_qkT((2, 6, 3, 7), ps_dots, N)   # heads 4-7, ps_dots slots
        if idx == 6:
            emit_vproj(ps_dots, N)
        if idx % GRP == 0:
            if 6 <= idx <= NTILES + 4:
                emit_B2(idx - 6)
            if 4 <= idx <= NTILES + 2:
                emit_B1(idx - 4)

    score_ctx.close()

    # ------- stage C: output projection + bias (fp16) -------
    with ExitStack() as fin:
        ps_out = fin.enter_context(tc.tile_pool(name="ps_out", bufs=2, space="PSUM"))
        yo_p = fin.enter_context(tc.tile_pool(name="yout", bufs=2))
        for i in range(NT):
            ps = ps_out.tile([P, DIM], F32)
            nc.tensor.matmul(ps[:], ones1[:, :], bout_sb[:, :], start=True, stop=False)
            for kt in range(KD):
                nc.tensor.matmul(
                    ps[:],
                    mergedT[:, kt, i * P : (i + 1) * P],
                    wout_sb[:, kt, :],
                    start=False,
                    stop=(kt == KD - 1),
                )
            yo = yo_p.tile([P, DIM], F32)
            nc.scalar.copy(yo[:], ps[:])
            nc.sync.dma_start(y[i * P : (i + 1) * P, :], yo[:])


_NC_CACHE = None


def _get_nc():
    global _NC_CACHE
    if _NC_CACHE is None:
        _NC_CACHE = _build_bass()
    return _NC_CACHE


def _in_maps(x, spd, head_mask, W_qkv, W_out, b_out):
    wqk = W_qkv[:, : 2 * INNER].copy()
    wqk[:, INNER:] *= SCALE                     # dots scale into k
    wqk = wqk.astype(np.float16)
    wv = np.ascontiguousarray(W_qkv[:, 2 * INNER :]).astype(np.float16)
    scale_m = head_mask * (H / head_mask.sum())  # head-dropout rescale
    wout = (W_out * np.repeat(scale_m, DH)[:, None]).astype(np.float16)
    bout = b_out.reshape(1, DIM).astype(np.float16)
    ones1 = np.ones((1, P), dtype=np.float16)

    maps = []
    for b in range(B):
        maps.append(
            {
                "xt": np.ascontiguousarray(x[b].T).astype(np.float16),
                "spd": np.ascontiguousarray(spd[b, 0]).astype(np.float16),
                "wqk": wqk,
                "wv": wv,
                "wout": wout,
                "bout": bout,
                "ones": ones1,
            }
        )
    return maps


def kernel(x, spd, head_mask, W_qkv, W_out, b_out):
    x = np.asarray(x, dtype=np.float32)
    spd = np.asarray(spd, dtype=np.float32)
    head_mask = np.asarray(head_mask, dtype=np.float32)
    W_qkv = np.asarray(W_qkv, dtype=np.float32)
    W_out = np.asarray(W_out, dtype=np.float32)
    b_out = np.asarray(b_out, dtype=np.float32)

    nc = _get_nc()
    res = run_bass_kernel_spmd(
        nc, _in_maps(x, spd, head_mask, W_qkv, W_out, b_out),
        core_ids=list(range(B)),
    )
    return np.stack([res.results[b]["y"] for b in range(B)], axis=0)


# revision 69
# speedup vs baseline: 1.0100x; 1.0100x over previous
"""Trainium2 Bass kernel for nn_Attention_spd (dense transformer attention
with spd-modulated bias), sharded batch-parallel across 8 NeuronCores.

Per batch b (one core each):
    qkv = x @ W_qkv ; q,k,v heads of 64
    dots = q @ k^T * DH**-0.5                       [h, n, m]
    bias = normalize(dots*spd, axis=-1) * ||dots||  (row-wise L2)
    attn = softmax(dots + bias) * head_mask_scale
    out  = (attn @ v) merged @ W_out + b_out

Identities used on device:
    z    = dots * (1 + spd * rho[n]),  rho = ||dots|| / max(||dots*spd||, eps)
    u[n] = sum_m dots^2   via ACT Square accum_out (rides the d2 pass)
    ssq  = sum_m d2 * spd^2  via DVE STT accum_out (spd^2 from host)
    rho  = exp(0.5*(ln(u+eps) - ln(ssq+eps)))  (single ACT table set)
    softmax denominator via ACT-exp accumulate; 1/den applied to E.

Structure: dots for each (head, n-tile) is computed ONCE into PSUM and
immediately copied to fp16 SBUF (copy duty alternates DVE/ACT to balance
load), freeing the PSUM bank so the PE never stalls. A software pipeline
emits stage A (dots/copy/d2/ssq) 4 tiles ahead of stage B1 (rho/w/z/exp)
and 6 ahead of B2 (1/den scale + transpose + EV), so each in-order engine
queue always holds independent big ops ahead of the cross-engine small-op
chains. Steady state runs the Vector engine at ~100% occupancy.

Precision plan: fp16 x/Wqk/spd and fp16 matmuls everywhere (1 cyc/col on
the PE); dots accumulated in fp32 PSUM; E in bf16 (range safety); E/v
matmul bf16. Host folds DH**-0.5 into the k columns of W_qkv and
head_mask * H/sum(mask) into the rows of W_out; x passed pre-transposed,
spd passed with its elementwise square.
"""

import numpy as np
from contextlib import ExitStack

import concourse.bass as bass
import concourse.tile as tile
import concourse.mybir as mybir
from concourse.bass_utils import run_bass_kernel_spmd
from concourse.vector_clock import ScopedClock

# ---------------- problem constants (hardcoded) ----------------
B, N, DIM, H, DH = 8, 1024, 512, 8, 64
INNER = H * DH            # 512
SCALE = DH ** -0.5
P = 128                   # SBUF partitions
NT = N // P               # 8 n-tiles (and m-tiles)
KD = DIM // P             # 4 contraction subtiles over DIM
AF = mybir.ActivationFunctionType
ALU = mybir.AluOpType
F32 = mybir.dt.float32
F16 = mybir.dt.float16
BF16 = mybir.dt.bfloat16

# ---------------- walrus workaround patches ----------------
# The walrus build in this container rejects instructions with more than one
# sync-wait command. Split excess waits onto same-engine NoOps, and spread
# the kernel-tail Drain's waits over extra Drains.
_MAX_WAITS = 1
_SKIP_TYPES = (
    "TileBranchInst",
    "BassTileLoopBlock",
    "BassTileConditionalBlock",
    "BassTileCriticalSection",
)


def _split_waits(nc, ordered):
    for _bb, insts in ordered.items():
        new_list = []
        for inst in insts:
            si = getattr(inst, "sync_info", None)
            if (
                si is not None
                and si.on_wait
                and len(si.on_wait) > _MAX_WAITS
                and type(inst).__name__ not in _SKIP_TYPES
            ):
                waits = list(si.on_wait)
                extra, keep = waits[:-_MAX_WAITS], waits[-_MAX_WAITS:]
                for j in range(0, len(extra), _MAX_WAITS):
                    nop = mybir.InstNoOp(
                        name=nc.get_next_instruction_name(),
                        sync_info=mybir.SyncInfo(
                            on_wait=extra[j : j + _MAX_WAITS], on_update=[]
                        ),
                        bass_nofuse=True,
                        engine=inst.engine,
                    )
                    new_list.append(nop)
                inst.sync_info = mybir.SyncInfo(on_wait=keep, on_update=si.on_update)
            new_list.append(inst)
        insts[:] = new_list


_orig_lower = tile.TileContext._lower_ordered_insts


def _patched_lower(self, ordered):
    _split_waits(self.nc, ordered)
    return _orig_lower(self, ordered)


def _patched_drain_and_barrier(self, tick_clock, wait_clock):
    nc = self.nc
    drain_inst = nc.sync.drain()
    wait_clock.add_sem_waits(
        drain_inst.ins, ScopedClock({None: tick_clock.global_clock})
    )
    waits = list(drain_inst.ins.sync_info.on_wait)
    if len(waits) > 1:
        drain_inst.ins.sync_info = mybir.SyncInfo(on_wait=waits[:1], on_update=[])
        for w in waits[1:]:
            extra = nc.sync.drain()
            extra.ins.sync_info = mybir.SyncInfo(on_wait=[w], on_update=[])
    nc.all_engine_barrier()
    assert self.sems is not None
    popped = nc._tile_sem_poison_stack.pop()
    assert popped is self._sem_poison
    nc.clear_and_free_semaphores(list(self.sems.allocated().values()))
    nc.all_engine_barrier()


def _apply_patches():
    if tile.TileContext._lower_ordered_insts is not _patched_lower:
        tile.TileContext._lower_ordered_insts = _patched_lower
        tile.TileContext._drain_and_barrier = _patched_drain_and_barrier


# ---------------- device kernel ----------------

def _build_bass():
    _apply_patches()
    nc = bass.Bass(
        "TRN2", target_bir_lowering=False, debug=False, enable_asserts=False
    )
    xt = nc.dram_tensor("xt", (DIM, N), F16, kind="ExternalInput").ap()
    spd = nc.dram_tensor("spd", (N, N), F16, kind="ExternalInput").ap()
    spd2 = nc.dram_tensor("spd2", (N, N), F16, kind="ExternalInput").ap()
    wqk = nc.dram_tensor("wqk", (DIM, 2 * INNER), F16, kind="ExternalInput").ap()
    wv = nc.dram_tensor("wv", (DIM, INNER), F16, kind="ExternalInput").ap()
    wout = nc.dram_tensor("wout", (INNER, DIM), F16, kind="ExternalInput").ap()
    bout = nc.dram_tensor("bout", (1, DIM), F16, kind="ExternalInput").ap()
    ones = nc.dram_tensor("ones", (1, P), F16, kind="ExternalInput").ap()
    y = nc.dram_tensor("y", (N, DIM), F32, kind="ExternalOutput").ap()

    with tile.TileContext(nc) as tc, ExitStack() as ctx:
        _emit(nc, tc, ctx, xt, spd, spd2, wqk, wv, wout, bout, ones, y)
    return nc


def _emit(nc, tc, ctx, xt, spd, spd2, wqk, wv, wout, bout, ones, y):
    # ------- persistent pools -------
    const_p = ctx.enter_context(tc.tile_pool(name="const", bufs=1))
    spd_p = ctx.enter_context(tc.tile_pool(name="spd", bufs=1))
    spd2_p = ctx.enter_context(tc.tile_pool(name="spd2", bufs=1))
    qkT_p = ctx.enter_context(tc.tile_pool(name="qkT", bufs=1))
    v_p = ctx.enter_context(tc.tile_pool(name="v", bufs=1))
    wout_p = ctx.enter_context(tc.tile_pool(name="wout", bufs=1))
    merged_p = ctx.enter_context(tc.tile_pool(name="merged", bufs=1))
    st_p = ctx.enter_context(tc.tile_pool(name="stats", bufs=1))

    ones1 = const_p.tile([1, P], F16)
    nc.sync.dma_start(ones1[:], ones[:])
    bout_sb = const_p.tile([1, DIM], F16)
    nc.sync.dma_start(bout_sb[:], bout[:])
    eps_t = const_p.tile([P, 1], F32, tag="eps")
    nc.vector.memset(eps_t[:], 1e-24)

    spd_sb = spd_p.tile([P, NT, N], F16)
    spd2_sb = spd2_p.tile([P, NT, N], F16)

    # qkT_sb[p, ft, n] = (x @ Wqk)^T : f = ft*128+p ; q is ft 0..3, k is ft 4..7
    qkT_sb = qkT_p.tile([P, 2 * INNER // P, N], F16)
    # v_sb[p, mt, :] = v[m, :] with m = mt*128+p (bf16 for the EV matmul)
    v_sb = v_p.tile([P, NT, INNER], BF16)
    wout_sb = wout_p.tile([P, KD, DIM], F16)
    nc.sync.dma_start(wout_sb[:], wout.rearrange("(kt p) d -> p kt d", p=P))
    # mergedT[p, s, n]: inner = s*128 + p  (head pair s = (2s, 2s+1))
    mergedT = merged_p.tile([P, KD, N], F16)

    # per-(head, n-tile) row stats; index hi = h*NT + i.
    # ssq (slot 0) and u (slot 1) interleaved so one Ln covers a pair.
    su_all = st_p.tile([P, H * NT, 2], F32, tag="su")
    lsu_all = st_p.tile([P, H * NT, 2], F32, tag="lsu")
    rho_all = st_p.tile([P, H * NT], F32, tag="rho")
    den_all = st_p.tile([P, H * NT], F32, tag="den")
    rden_all = st_p.tile([P, H * NT], F32, tag="rden")

    # ------- stage A: projections (all fp16 matmuls) -------
    with ExitStack() as early:
        xt_p = early.enter_context(tc.tile_pool(name="xt", bufs=1))
        wqk_p = early.enter_context(tc.tile_pool(name="wqk", bufs=1))
        wv_p = early.enter_context(tc.tile_pool(name="wv", bufs=1))
        ps_proj = early.enter_context(
            tc.tile_pool(name="ps_proj", bufs=4, space="PSUM")
        )

        xt_sb = xt_p.tile([P, KD, N], F16)
        nc.sync.dma_start(xt_sb[:], xt.rearrange("(kt p) n -> p kt n", p=P))
        wqk_sb = wqk_p.tile([P, KD, 2 * INNER], F16)
        nc.sync.dma_start(wqk_sb[:], wqk.rearrange("(kt p) f -> p kt f", p=P))
        wv_sb = wv_p.tile([P, KD, INNER], F16)
        # spd2/spd after the projection inputs: first needed at ssq/w of tile 0;
        # wv only at the first EV (~16 tiles in)
        nc.sync.dma_start(spd2_sb[:], spd2.rearrange("(t p) m -> p t m", p=P))
        nc.sync.dma_start(spd_sb[:], spd.rearrange("(t p) m -> p t m", p=P))
        nc.sync.dma_start(wv_sb[:], wv.rearrange("(kt p) f -> p kt f", p=P))

        # qkT: head-pair-major ft order so head 0/1 scores can start early
        cpy = 0
        for ft in (0, 4, 1, 5, 2, 6, 3, 7):
            for ch in range(2):
                ps = ps_proj.tile([P, 512], F32)
                for kt in range(KD):
                    nc.tensor.matmul(
                        ps[:],
                        wqk_sb[:, kt, ft * P : (ft + 1) * P],
                        xt_sb[:, kt, ch * 512 : (ch + 1) * 512],
                        start=(kt == 0),
                        stop=(kt == KD - 1),
                    )
                dst = qkT_sb[:, ft, ch * 512 : (ch + 1) * 512]
                if cpy % 2 == 0:
                    nc.vector.tensor_copy(dst, ps[:])
                else:
                    nc.scalar.copy(dst, ps[:])
                cpy += 1

        # v (bf16 out)
        for mt in range(NT):
            ps = ps_proj.tile([P, 512], F32)
            for kt in range(KD):
                nc.tensor.matmul(
                    ps[:],
                    xt_sb[:, kt, mt * P : (mt + 1) * P],
                    wv_sb[:, kt, :],
                    start=(kt == 0),
                    stop=(kt == KD - 1),
                )
            if mt % 2 == 0:
                nc.vector.tensor_copy(v_sb[:, mt, :], ps[:])
            else:
                nc.scalar.copy(v_sb[:, mt, :], ps[:])

    # ------- stage B: scores per head, single dots pass -------
    score_ctx = ExitStack()
    ctx.enter_context(score_ctx)
    et_p = score_ctx.enter_context(tc.tile_pool(name="ET", bufs=2))
    dc_p = score_ctx.enter_context(tc.tile_pool(name="dc", bufs=10))
    d2_p = score_ctx.enter_context(tc.tile_pool(name="d2", bufs=5))
    junk_p = score_ctx.enter_context(tc.tile_pool(name="sjunk", bufs=2))
    w_p = score_ctx.enter_context(tc.tile_pool(name="wbuf", bufs=5))
    z_p = score_ctx.enter_context(tc.tile_pool(name="zbuf", bufs=5))
    e_p = score_ctx.enter_context(tc.tile_pool(name="E", bufs=8))
    ln_p = score_ctx.enter_context(tc.tile_pool(name="lnjunk", bufs=4))
    rd_p = score_ctx.enter_context(tc.tile_pool(name="rden", bufs=2))
    ps_dots = score_ctx.enter_context(
        tc.tile_pool(name="ps_dots", bufs=3, space="PSUM")
    )
    ps_ev = score_ctx.enter_context(tc.tile_pool(name="ps_ev", bufs=2, space="PSUM"))

    et_tiles = {}
    dcs = {}
    zbufs = {}
    ebufs = {}

    def qk_mm(psd, h, i):
        base = (h % 2) * DH
        for ch in range(2):
            nc.tensor.matmul(
                psd[:, ch * 512 : (ch + 1) * 512],
                qkT_sb[base : base + DH, h // 2, i * P : (i + 1) * P],
                qkT_sb[base : base + DH, 4 + h // 2, ch * 512 : (ch + 1) * 512],
                start=True,
                stop=True,
            )

    TILES = [(h, i) for h in range(H) for i in range(NT)]

    def emit_A(t):
        h, i = TILES[t]
        hi = h * NT + i
        psd = ps_dots.tile([P, N], F32)
        qk_mm(psd, h, i)
        # copy dots out of PSUM right away (frees the bank for the PE);
        # alternate the PSUM-read duty between DVE and ACT to balance load
        dc = dc_p.tile([P, N], F16)
        dcs[t] = dc
        d2 = d2_p.tile([P, N], F16)
        if t % 2 == 0:
            nc.scalar.copy(dc[:], psd[:])
            nc.scalar.activation(
                d2[:], psd[:], AF.Square, accum_out=su_all[:, hi, 1:2]
            )
        else:
            nc.vector.tensor_copy(dc[:], psd[:])
            nc.scalar.activation(
                d2[:], dc[:], AF.Square, accum_out=su_all[:, hi, 1:2]
            )
        # d2*spd2 at TT 2x, then the row-sum via 4x tensor_scalar accum
        # (STT would fuse both but only has a 1x uop); both TS ops must be
        # populated or the BIR verifier rejects the reduce form
        junk = junk_p.tile([P, N], F16, tag="j1")
        nc.vector.tensor_tensor(junk[:], d2[:], spd2_sb[:, i, :], ALU.mult)
        junk2 = junk_p.tile([P, N], F16, tag="j2")
        nc.vector.tensor_scalar(
            junk2[:], junk[:], 1.0, 0.0, ALU.mult, ALU.add,
            accum_out=su_all[:, hi, 0:1],
        )

    GRP = 2

    def emit_B1(t0):
        # rho for the pair (t0, t0+1), then w, z, exp+den
        h, i0 = TILES[t0]
        lo = h * NT + i0
        sl = slice(lo, lo + GRP)
        nc.scalar.activation(
            lsu_all[:, sl, :], su_all[:, sl, :], AF.Ln, bias=eps_t[:, 0:1]
        )
        s4 = ln_p.tile([P, GRP], F32, tag="s4")
        nc.vector.tensor_tensor(
            s4[:], lsu_all[:, sl, 1], lsu_all[:, sl, 0], ALU.subtract
        )
        nc.scalar.activation(rho_all[:, sl], s4[:], AF.Exp, scale=0.5)
        for t in range(t0, t0 + GRP):
            h, i = TILES[t]
            hi = h * NT + i
            w16 = w_p.tile([P, N], F16)
            nc.vector.tensor_scalar(
                w16[:],
                spd_sb[:, i, :],
                rho_all[:, hi : hi + 1],
                1.0,
                ALU.mult,
                ALU.add,
            )
            z16 = z_p.tile([P, N], F16)
            nc.vector.tensor_tensor(z16[:], dcs.pop(t)[:], w16[:], ALU.mult)
            zbufs[t] = z16
        for t in range(t0, t0 + GRP):
            h, i = TILES[t]
            hi = h * NT + i
            eb = e_p.tile([P, N], BF16)
            ebufs[t] = eb
            nc.scalar.activation(
                eb[:], zbufs.pop(t)[:], AF.Exp, accum_out=den_all[:, hi : hi + 1]
            )

    def emit_B2(t0):
        # transpose raw (unnormalized) E out; EV when a head pair closes.
        # 1/den is folded into the EV psum->merged multiply via R.
        h, i0 = TILES[t0]
        et = et_tiles.get(h)
        if et is None:
            et = et_p.tile([P, NT, N], BF16)
            et_tiles[h] = et
        lo = h * NT + i0
        sl = slice(lo, lo + GRP)
        nc.vector.reciprocal(rden_all[:, sl], den_all[:, sl])
        for t in range(t0, t0 + GRP):
            _, i = TILES[t]
            hi = h * NT + i
            eb = ebufs.pop(t)
            nc.vector.tensor_scalar_mul(eb[:], eb[:], rden_all[:, hi : hi + 1])
            nc.sync.dma_start_transpose(et[:, :, i * P : (i + 1) * P], eb[:])
        if i0 + GRP == NT and h % 2 == 1:
            for ch in range(2):
                ps = ps_ev.tile([P, 512], F32)
                for hh in (h - 1, h):
                    bb = (hh % 2) * DH
                    for mt in range(NT):
                        nc.tensor.matmul(
                            ps[bb : bb + DH, :],
                            v_sb[:, mt, hh * DH : (hh + 1) * DH],
                            et_tiles[hh][:, mt, ch * 512 : (ch + 1) * 512],
                            start=(mt == 0),
                            stop=(mt == NT - 1),
                            tile_position=(0, bb),
                        )
                nc.scalar.copy(
                    mergedT[:, h // 2, ch * 512 : (ch + 1) * 512], ps[:]
                )
            et_tiles.clear()

    # software pipeline: A runs 2 tiles ahead of B1, 4 ahead of B2, so each
    # engine's in-order queue always holds independent big ops ahead of the
    # cross-engine rho/den small-op chains.
    NTILES = len(TILES)
    for idx in range(NTILES + 7):
        if idx < NTILES:
            emit_A(idx)
        if idx % GRP == 0:
            if 6 <= idx <= NTILES + 4:
                emit_B2(idx - 6)
            if 4 <= idx <= NTILES + 2:
                emit_B1(idx - 4)

    score_ctx.close()

    # ------- stage C: output projection + bias (fp16) -------
    with ExitStack() as fin:
        ps_out = fin.enter_context(tc.tile_pool(name="ps_out", bufs=2, space="PSUM"))
        yo_p = fin.enter_context(tc.tile_pool(name="yout", bufs=2))
        for i in range(NT):
            ps = ps_out.tile([P, DIM], F32)
            nc.tensor.matmul(ps[:], ones1[:, :], bout_sb[:, :], start=True, stop=False)
            for kt in range(KD):
                nc.tensor.matmul(
                    ps[:],
                    mergedT[:, kt, i * P : (i + 1) * P],
                    wout_sb[:, kt, :],
                    start=False,
                    stop=(kt == KD - 1),
                )
            yo = yo_p.tile([P, DIM], F32)
            nc.scalar.copy(yo[:], ps[:])
            nc.sync.dma_start(y[i * P : (i + 1) * P, :], yo[:])


_NC_CACHE = None


def _get_nc():
    global _NC_CACHE
    if _NC_CACHE is None:
        _NC_CACHE = _build_bass()
    return _NC_CACHE


def _in_maps(x, spd, head_mask, W_qkv, W_out, b_out):
    wqk = W_qkv[:, : 2 * INNER].copy()
    wqk[:, INNER:] *= SCALE                     # dots scale into k
    wqk = wqk.astype(np.float16)
    wv = np.ascontiguousarray(W_qkv[:, 2 * INNER :]).astype(np.float16)
    scale_m = head_mask * (H / head_mask.sum())  # head-dropout rescale
    wout = (W_out * np.repeat(scale_m, DH)[:, None]).astype(np.float16)
    bout = b_out.reshape(1, DIM).astype(np.float16)
    ones1 = np.ones((1, P), dtype=np.float16)

    maps = []
    for b in range(B):
        maps.append(
            {
                "xt": np.ascontiguousarray(x[b].T).astype(np.float16),
                "spd": np.ascontiguousarray(spd[b, 0]).astype(np.float16),
                "wqk": wqk,
                "wv": wv,
                "wout": wout,
                "bout": bout,
                "ones": ones1,
            }
        )
    return maps


def kernel(x, spd, head_mask, W_qkv, W_out, b_out):
    x = np.asarray(x, dtype=np.float32)
    spd = np.asarray(spd, dtype=np.float32)
    head_mask = np.asarray(head_mask, dtype=np.float32)
    W_qkv = np.asarray(W_qkv, dtype=np.float32)
    W_out = np.asarray(W_out, dtype=np.float32)
    b_out = np.asarray(b_out, dtype=np.float32)

    nc = _get_nc()
    res = run_bass_kernel_spmd(
        nc, _in_maps(x, spd, head_mask, W_qkv, W_out, b_out),
        core_ids=list(range(B)),
    )
    return np.stack([res.results[b]["y"] for b in range(B)], axis=0)
